# revision 1
# baseline (speedup 1.0000x reference)
"""2-layer GCN (PyG GCNConv + BN + ReLU) on 8 Trainium2 NeuronCores.

Strategy (node sharding per spec hint):
  - Nodes sorted by in-degree (desc) and dealt round-robin to 8 cores;
    each core owns S node slots (real nodes + dead all-zero rows).
  - Per layer: each core computes hs = (a @ W) * dinv for its shard
    ([node,f] rows for the gather table, [f,node]*dinv^2 for the self-loop
    term), then AllGather -> full hs table [G,128] in every core's DRAM.
  - Edges partitioned by destination core, grouped into 256-dst windows;
    per 128-edge tile: dma_gather source rows, build one-hot
    P[slot,dst]=(dstloc==iota) on DVE, accumulate M^T @ P into PSUM
    [f=128, dst=256] with float32r matmuls (1 cyc/row at free>=256).
  - pre = psum*dinv[dst] + h[dst]*dinv^2; BN stats via free-dim reduces +
    a tiny AllReduce; y = relu(pre*s + t) on ACT (per-partition scale/bias).
  - Output returned transposed [128, S] per core; host reassembles.

dma_gather indices are int16 (<32768), so the G-row table is addressed
through two overlapping views: lo=[0,32768) and hi=[G-32768,G); padding
slots point at dead (all-zero) rows owned by core 0 / core 7.
b1/b2 are dropped: BN immediately follows the +b and is invariant to
per-feature constant shifts.
"""

import numpy as np

C = 8            # cores
D = 128          # feature dim
WIN = 256        # dst nodes per aggregation window (psum free dim)
CH = 8           # max gather tiles (of 128 slots) per dma_gather call
IDX_LIMIT = 32768

_cache = {}


def _plan(edge_index, N):
    """Host-side graph preprocessing -> per-core arrays + static structure."""
    src = np.asarray(edge_index[0], dtype=np.int64)
    dst = np.asarray(edge_index[1], dtype=np.int64)
    E = src.shape[0]

    Nr = -(-N // C)                      # real nodes per core
    S = -(-Nr // WIN) * WIN              # padded slots per core
    NW = S // WIN
    G = C * S                            # table rows
    assert G > IDX_LIMIT and S - Nr >= 1, (G, S, Nr)

    deg = np.bincount(dst, minlength=N).astype(np.int64) + 1
    order = np.argsort(-deg, kind="stable")        # rank -> old id
    ranks = np.arange(N, dtype=np.int64)
    g_of_old = np.empty(N, dtype=np.int64)
    g_of_old[order] = (ranks % C) * S + ranks // C

    dinv = np.zeros(G, dtype=np.float64)
    dinv[g_of_old] = deg.astype(np.float64) ** -0.5

    gs = g_of_old[src]
    gd = g_of_old[dst]
    core_e = gd // S
    w_e = (gd % S) // WIN
    dl_e = (gd % S) % WIN
    view_e = (gs >= IDX_LIMIT).astype(np.int64)
    idx_e = np.where(view_e == 0, gs, gs - (G - IDX_LIMIT))
    assert idx_e.max() < IDX_LIMIT and idx_e.min() >= 0

    pad_idx = (S - 1, IDX_LIMIT - 1)     # core0 dead row (lo) / core7 dead (hi)

    counts = np.zeros((C, NW, 2), dtype=np.int64)
    np.add.at(counts, (core_e, w_e, view_e), 1)
    K = -(-counts.max(axis=0) // 128)             # [NW, 2] unified tile counts

    # static tile list in slot order: for each w: [lo tiles][hi tiles]
    tile_w, tile_v = [], []
    for w in range(NW):
        tile_w += [w] * int(K[w, 0]); tile_v += [0] * int(K[w, 0])
        tile_w += [w] * int(K[w, 1]); tile_v += [1] * int(K[w, 1])
    tile_w = np.array(tile_w, dtype=np.int64)
    tile_v = np.array(tile_v, dtype=np.int64)
    TOTK = len(tile_w)
    TOTS = TOTK * 128

    chunks = []   # (tile0, ntiles, view) — runs of one view, <= CH tiles
    t = 0
    while t < TOTK:
        v = tile_v[t]
        r = t
        while r < TOTK and tile_v[r] == v and r - t < CH:
            r += 1
        chunks.append((int(t), int(r - t), int(v)))
        t = r
    first_tile, last_tile = {}, {}
    for t in range(TOTK):
        w = int(tile_w[t])
        first_tile.setdefault(w, t)
        last_tile[w] = t

    # slot base of each (w, v) segment
    tile_base = np.zeros((NW, 2), dtype=np.int64)
    b = 0
    for w in range(NW):
        for v in range(2):
            tile_base[w, v] = b
            b += int(K[w, v]) * 128
    slot_view = np.empty(TOTS, dtype=np.int64)
    for w in range(NW):
        for v in range(2):
            s0 = int(tile_base[w, v]); n = int(K[w, v]) * 128
            slot_view[s0:s0 + n] = v

    ord_e = np.lexsort((dl_e, view_e, w_e, core_e))
    src_s = idx_e[ord_e]; core_s = core_e[ord_e]
    w_s = w_e[ord_e]; v_s = view_e[ord_e]; dl_s = dl_e[ord_e]

    idx_all = np.empty((C, TOTS), dtype=np.int16)
    dst_all = np.zeros((C, TOTS), dtype=np.float32)
    for c in range(C):
        m = core_s == c
        iw, iv, ii, idl = w_s[m], v_s[m], src_s[m], dl_s[m]
        arr_i = np.full(TOTS, -1, dtype=np.int64)
        arr_d = np.zeros(TOTS, dtype=np.int64)
        if len(iw):
            seg_id = iw * 2 + iv
            change = np.r_[True, np.diff(seg_id) != 0]
            seg_start = np.maximum.accumulate(
                np.where(change, np.arange(len(seg_id)), 0))
            within = np.arange(len(seg_id)) - seg_start
            flat = tile_base[iw, iv] + within
            arr_i[flat] = ii
            arr_d[flat] = idl
        padm = arr_i == -1
        arr_i[padm] = np.where(slot_view[padm] == 0, pad_idx[0], pad_idx[1])
        idx_all[c] = arr_i.astype(np.int16)
        dst_all[c] = arr_d.astype(np.float32)

    # dma_gather idx wrap: [128, TOTS/16] int16, 16-row pattern tiled x8
    idx_wrap = np.empty((C, 128, TOTS // 16), dtype=np.int16)
    for c in range(C):
        idx_wrap[c] = np.tile(idx_all[c].reshape(-1, 16).T, (8, 1))
    # dstloc tile layout [128, TOTK]: [p, t] = dst_local of slot t*128+p
    dstloc = dst_all.reshape(C, TOTK, 128).transpose(0, 2, 1).copy()

    dinv_f = dinv.astype(np.float32).reshape(C, S)
    dinvT = np.broadcast_to(dinv_f[:, None, :], (C, 128, S)).copy()
    dinv_cols = dinv_f.reshape(C, S // 128, 128).transpose(0, 2, 1).copy()

    return dict(
        N=N, E=E, S=S, NW=NW, G=G, TOTK=TOTK, TOTS=TOTS,
        K=K, chunks=chunks, tile_w=tile_w, tile_v=tile_v,
        first_tile=first_tile, last_tile=last_tile,
        g_of_old=g_of_old, idx_wrap=idx_wrap, dstloc=dstloc,
        dinvT=dinvT, dinv_cols=dinv_cols,
    )


def _build(tc, outs, ins, plan):
    """Emit the Tile program. ins/outs: dicts of DRAM APs."""
    import contextlib

    import concourse.mybir as mybir

    nc = tc.nc
    S, NW, G, TOTK = plan["S"], plan["NW"], plan["G"], plan["TOTK"]
    N = plan["N"]
    HN = S // 128
    f32, f32r = mybir.dt.float32, mybir.dt.float32r
    i16 = mybir.dt.int16
    AF = mybir.ActivationFunctionType
    OP = mybir.AluOpType
    rg = [list(range(C))]

    hs_dram = [nc.dram_tensor(f"hs{l}", [S, D], f32) for l in range(2)]
    table = [nc.dram_tensor(f"table{l}", [G, D], f32, addr_space="Shared")
             for l in range(2)]
    bnin = [nc.dram_tensor(f"bnin{l}", [128, 2], f32) for l in range(2)]
    bnout = [nc.dram_tensor(f"bnout{l}", [128, 2], f32, addr_space="Shared")
             for l in range(2)]

    ctx = contextlib.ExitStack()
    with ctx:
        persist = ctx.enter_context(tc.tile_pool(name="persist", bufs=1))
        gpool = ctx.enter_context(tc.tile_pool(name="gather", bufs=3))
        ppool = ctx.enter_context(tc.tile_pool(name="ptiles", bufs=4))
        spool = ctx.enter_context(tc.tile_pool(name="scratch", bufs=3))
        rowp = ctx.enter_context(tc.tile_pool(name="rows", bufs=3))
        psum_agg = ctx.enter_context(tc.tile_pool(name="psagg", bufs=4, space="PSUM"))
        psum_mm = ctx.enter_context(tc.tile_pool(name="psmm", bufs=2, space="PSUM"))

        aT = persist.tile([128, S], f32, tag="aT")       # a0T -> a1T -> apre2
        apre = persist.tile([128, S], f32, tag="apre")   # apre1 -> a2T
        hsT2 = persist.tile([128, S], f32, tag="hsT2")
        dinvT = persist.tile([128, S], f32, tag="dinvT")
        dloc = persist.tile([128, TOTK], f32, tag="dloc")
        idxs = persist.tile([128, plan["TOTS"] // 16], i16, tag="idxs")
        iota = persist.tile([128, WIN], f32, tag="iota")
        Wt = [persist.tile([128, D], f32, tag=f"W{l}", name=f"Wt{l}") for l in range(2)]
        dcols = persist.tile([128, HN], f32, tag="dcols")
        gbs = persist.tile([128, 4], f32, tag="gbs")
        stats = persist.tile([128, 2 * NW], f32, tag="stats")  # [0:NW]=sum, [NW:]=sumsq
        bnsb = [persist.tile([128, 2], f32, tag=f"bnsb{l}", name=f"bnsb{l}") for l in range(2)]
        svec = [persist.tile([128, 1], f32, tag=f"svec{l}", name=f"svec{l}") for l in range(2)]
        tvec = [persist.tile([128, 1], f32, tag=f"tvec{l}", name=f"tvec{l}") for l in range(2)]

        nc.sync.dma_start(out=aT[:], in_=ins["xT"][:, :])
        nc.sync.dma_start(out=dinvT[:], in_=ins["dinvT"][:, :])
        nc.sync.dma_start(out=dloc[:], in_=ins["dstloc"][:, :])
        nc.sync.dma_start(out=idxs[:], in_=ins["idxw"][:, :])
        nc.sync.dma_start(out=iota[:], in_=ins["iota"][:, :])
        nc.sync.dma_start(out=dcols[:], in_=ins["dinv_cols"][:, :])
        nc.sync.dma_start(out=gbs[:], in_=ins["gb"][:, :])
        nc.sync.dma_start(out=Wt[0][:], in_=ins["W1"][:, :])
        nc.sync.dma_start(out=Wt[1][:], in_=ins["W2"][:, :])

        def phase_mm(l, src_tile):
            """hs{l} rows -> DRAM table input; hsT2 = (W^T a)*dinv^2 cols."""
            for h in range(HN):
                sl = slice(h * 128, (h + 1) * 128)
                psA = psum_mm.tile([128, D], f32, tag="psA")
                nc.tensor.matmul(out=psA[:], lhsT=src_tile[:, sl], rhs=Wt[l][:],
                                 start=True, stop=True)
                hrow = rowp.tile([128, D], f32, tag="hrow")
                nc.vector.tensor_scalar(out=hrow[:], in0=psA[:],
                                        scalar1=dcols[:, h:h + 1], scalar2=None,
                                        op0=OP.mult)
                nc.sync.dma_start(out=hs_dram[l][sl, :], in_=hrow[:])
                psB = psum_mm.tile([128, D], f32, tag="psB")
                nc.tensor.matmul(out=psB[:], lhsT=Wt[l][:], rhs=src_tile[:, sl],
                                 start=True, stop=True)
                tmp = rowp.tile([128, D], f32, tag="tmpB")
                nc.vector.tensor_tensor(out=tmp[:], in0=psB[:], in1=dinvT[:, sl],
                                        op=OP.mult)
                nc.vector.tensor_tensor(out=hsT2[:, sl], in0=tmp[:],
                                        in1=dinvT[:, sl], op=OP.mult)

        def win_epilogue(w, psw_w, pre_tile):
            wsl = slice(w * WIN, (w + 1) * WIN)
            tmp = spool.tile([128, WIN], f32, tag="ep")
            nc.vector.tensor_tensor(out=tmp[:], in0=psw_w[:],
                                    in1=dinvT[:, wsl], op=OP.mult)
            nc.vector.tensor_tensor(out=pre_tile[:, wsl], in0=tmp[:],
                                    in1=hsT2[:, wsl], op=OP.add)
            nc.vector.tensor_reduce(out=stats[:, w:w + 1], in_=pre_tile[:, wsl],
                                    axis=mybir.AxisListType.X, op=OP.add)
            sq = spool.tile([128, WIN], f32, tag="sq")
            nc.scalar.activation(out=sq[:], in_=pre_tile[:, wsl], func=AF.Square,
                                 accum_out=stats[:, NW + w:NW + w + 1])

        def phase_agg(l, pre_tile):
            lo_ap = table[l][0:IDX_LIMIT, :]
            hi_ap = table[l][G - IDX_LIMIT:G, :]
            psw = {}
            for (t0, nt, v) in plan["chunks"]:
                n_idx = nt * 128
                gb = gpool.tile([128, CH * 128], f32r, tag="gbuf")
                out3d = gb[:, :n_idx].rearrange("p (k f) -> p k f", f=D)
                nc.gpsimd.dma_gather(
                    out_ap=out3d,
                    in_ap=(lo_ap if v == 0 else hi_ap).bitcast(f32r),
                    idxs_ap=idxs[:, t0 * 8:(t0 + nt) * 8],
                    num_idxs=n_idx, num_idxs_reg=n_idx, elem_size=D,
                )
                for k in range(nt):
                    t = t0 + k
                    w = int(plan["tile_w"][t])
                    if plan["first_tile"][w] == t:
                        psw[w] = psum_agg.tile([128, WIN], f32, tag="psw", name=f"psw{w}")
                    P = ppool.tile([128, WIN], f32r, tag="P")
                    nc.vector.tensor_tensor(
                        out=P[:], in0=dloc[:, t:t + 1].to_broadcast([128, WIN]),
                        in1=iota[:], op=OP.is_equal)
                    nc.tensor.matmul(
                        out=psw[w][:],
                        lhsT=gb[:, k * 128:(k + 1) * 128],
                        rhs=P[:],
                        start=(plan["first_tile"][w] == t),
                        stop=(plan["last_tile"][w] == t))
                    if plan["last_tile"][w] == t:
                        win_epilogue(w, psw.pop(w), pre_tile)
            for w in range(NW):           # windows with no edges at all
                if w not in plan["first_tile"]:
                    wsl = slice(w * WIN, (w + 1) * WIN)
                    nc.vector.tensor_copy(out=pre_tile[:, wsl], in_=hsT2[:, wsl])
                    nc.vector.tensor_reduce(out=stats[:, w:w + 1],
                                            in_=pre_tile[:, wsl],
                                            axis=mybir.AxisListType.X, op=OP.add)
                    sq = spool.tile([128, WIN], f32, tag="sq")
                    nc.scalar.activation(out=sq[:], in_=pre_tile[:, wsl],
                                         func=AF.Square,
                                         accum_out=stats[:, NW + w:NW + w + 1])

        def phase_bn(l, pre_tile, dst_tile):
            ssum = spool.tile([128, 1], f32, tag="bns")
            ssq = spool.tile([128, 1], f32, tag="bnq")
            nc.vector.tensor_reduce(out=ssum[:], in_=stats[:, 0:NW],
                                    axis=mybir.AxisListType.X, op=OP.add)
            nc.vector.tensor_reduce(out=ssq[:], in_=stats[:, NW:2 * NW],
                                    axis=mybir.AxisListType.X, op=OP.add)
            pk = spool.tile([128, 2], f32, tag="bnpack")
            nc.vector.tensor_copy(out=pk[:, 0:1], in_=ssum[:])
            nc.vector.tensor_copy(out=pk[:, 1:2], in_=ssq[:])
            nc.sync.dma_start(out=bnin[l][:, :], in_=pk[:])
            nc.gpsimd.collective_compute(
                "AllReduce", OP.add, replica_groups=rg,
                ins=[bnin[l].ap()], outs=[bnout[l].ap()])
            nc.sync.dma_start(out=bnsb[l][:], in_=bnout[l][:, :])
            st = bnsb[l]
            mean = spool.tile([128, 1], f32, tag="mean")
            var = spool.tile([128, 1], f32, tag="var")
            nc.vector.tensor_scalar(out=mean[:], in0=st[:, 0:1],
                                    scalar1=1.0 / N, scalar2=None, op0=OP.mult)
            nc.vector.tensor_scalar(out=var[:], in0=st[:, 1:2],
                                    scalar1=1.0 / N, scalar2=None, op0=OP.mult)
            m2 = spool.tile([128, 1], f32, tag="m2")
            nc.vector.tensor_tensor(out=m2[:], in0=mean[:], in1=mean[:], op=OP.mult)
            nc.vector.tensor_sub(out=var[:], in0=var[:], in1=m2[:])
            nc.vector.tensor_scalar(out=var[:], in0=var[:], scalar1=1e-5,
                                    scalar2=None, op0=OP.add)
            sd = spool.tile([128, 1], f32, tag="sd")
            nc.scalar.sqrt(out=sd[:], in_=var[:])
            rsd = spool.tile([128, 1], f32, tag="rsd")
            nc.vector.reciprocal(out=rsd[:], in_=sd[:])
            nc.vector.tensor_tensor(out=svec[l][:], in0=gbs[:, 2 * l:2 * l + 1],
                                    in1=rsd[:], op=OP.mult)
            ms = spool.tile([128, 1], f32, tag="ms")
            nc.vector.tensor_tensor(out=ms[:], in0=mean[:], in1=svec[l][:],
                                    op=OP.mult)
            nc.vector.tensor_sub(out=tvec[l][:], in0=gbs[:, 2 * l + 1:2 * l + 2],
                                 in1=ms[:])
            for w in range(NW):
                wsl = slice(w * WIN, (w + 1) * WIN)
                nc.scalar.activation(out=dst_tile[:, wsl], in_=pre_tile[:, wsl],
                                     func=AF.Relu, bias=tvec[l][:],
                                     scale=svec[l][:])

        # layer 1
        phase_mm(0, aT)
        nc.gpsimd.collective_compute(
            "AllGather", OP.bypass, replica_groups=rg,
            ins=[hs_dram[0].ap()], outs=[table[0].ap()])
        phase_agg(0, apre)
        phase_bn(0, apre, aT)          # aT := a1T
        # layer 2
        phase_mm(1, aT)
        nc.gpsimd.collective_compute(
            "AllGather", OP.bypass, replica_groups=rg,
            ins=[hs_dram[1].ap()], outs=[table[1].ap()])
        phase_agg(1, aT)               # aT := apre2 (a1T dead after phase_mm)
        phase_bn(1, aT, apre)          # apre := a2T
        nc.sync.dma_start(out=outs["outT"][:, :], in_=apre[:])


def _make_inputs(plan, x, W1, W2, gamma1, beta1, gamma2, beta2):
    S, G = plan["S"], plan["G"]
    xg = np.zeros((G, D), dtype=np.float32)
    xg[plan["g_of_old"]] = np.asarray(x, dtype=np.float32)
    iota = np.tile(np.arange(WIN, dtype=np.float32), (128, 1))
    gb = np.stack([np.asarray(gamma1, np.float32), np.asarray(beta1, np.float32),
                   np.asarray(gamma2, np.float32), np.asarray(beta2, np.float32)],
                  axis=1).copy()
    in_maps = []
    for c in range(C):
        in_maps.append({
            "xT": np.ascontiguousarray(xg[c * S:(c + 1) * S].T),
            "idxw": plan["idx_wrap"][c],
            "dstloc": plan["dstloc"][c],
            "dinvT": plan["dinvT"][c],
            "dinv_cols": plan["dinv_cols"][c],
            "W1": np.ascontiguousarray(np.asarray(W1, np.float32)),
            "W2": np.ascontiguousarray(np.asarray(W2, np.float32)),
            "gb": gb, "iota": iota,
        })
    return in_maps


def _declare_io(nc, plan):
    import concourse.mybir as mybir
    f32, i16 = mybir.dt.float32, mybir.dt.int16
    S, TOTK, TOTS = plan["S"], plan["TOTK"], plan["TOTS"]
    ins = {
        "xT": nc.dram_tensor("xT", [128, S], f32, kind="ExternalInput"),
        "idxw": nc.dram_tensor("idxw", [128, TOTS // 16], i16, kind="ExternalInput"),
        "dstloc": nc.dram_tensor("dstloc", [128, TOTK], f32, kind="ExternalInput"),
        "dinvT": nc.dram_tensor("dinvT", [128, S], f32, kind="ExternalInput"),
        "dinv_cols": nc.dram_tensor("dinv_cols", [128, S // 128], f32,
                                    kind="ExternalInput"),
        "W1": nc.dram_tensor("W1", [128, D], f32, kind="ExternalInput"),
        "W2": nc.dram_tensor("W2", [128, D], f32, kind="ExternalInput"),
        "gb": nc.dram_tensor("gb", [128, 4], f32, kind="ExternalInput"),
        "iota": nc.dram_tensor("iota", [128, WIN], f32, kind="ExternalInput"),
    }
    outs = {"outT": nc.dram_tensor("outT", [128, S], f32, kind="ExternalOutput")}
    return ins, outs


def _compile(plan, trace_sim=False):
    from concourse import bacc, tile

    nc = bacc.Bacc("TRN2", target_bir_lowering=False, debug=False, num_devices=C)
    ins, outs = _declare_io(nc, plan)
    ins_ap = {k: v for k, v in ins.items()}
    outs_ap = {k: v for k, v in outs.items()}
    with tile.TileContext(nc, trace_sim=trace_sim) as tc:
        _build(tc, outs_ap, ins_ap, plan)
    nc.compile()
    return nc


def _run(x, edge_index, W1, b1, gamma1, beta1, W2, b2, gamma2, beta2,
         trace=False):
    from concourse.bass_utils import run_bass_kernel_spmd

    N = x.shape[0]
    plan = _plan(np.asarray(edge_index), N)
    key = (N, plan["E"], plan["TOTK"], tuple(int(k) for k in plan["K"].ravel()))
    if key not in _cache:
        _cache.clear()
        _cache[key] = _compile(plan)
    nc = _cache[key]
    in_maps = _make_inputs(plan, x, W1, W2, gamma1, beta1, gamma2, beta2)
    res = run_bass_kernel_spmd(nc, in_maps, core_ids=list(range(C)), trace=trace)
    S, G = plan["S"], plan["G"]
    aT_full = np.concatenate([res.results[c]["outT"] for c in range(C)], axis=1)
    assert aT_full.shape == (128, G)
    y = np.ascontiguousarray(aT_full.T[plan["g_of_old"]], dtype=np.float32)
    return y, res


def kernel(**inputs):
    y, _ = _run(**inputs)
    return y



# revision 21
# speedup vs baseline: 1.2001x; 1.2001x over previous
"""2-layer GCN (PyG GCNConv + BN + ReLU) on 8 Trainium2 NeuronCores.

Strategy (node sharding, bf16 data path):
  - Nodes sorted by in-degree (desc), dealt round-robin to 8 cores; each
    core owns S slots (real nodes + dead all-zero rows).
  - Per layer: each core computes hs = (a @ W) * dinv for its shard
    (matmul + ACT copy-with-scale into an SBUF [node,f] bf16 block and a
    DRAM copy), AllGather -> full bf16 table [G,128] in every core's DRAM.
  - Edges partitioned by destination core, grouped into WIN-dst windows;
    windows processed in PSUM-resident groups.  Per 128-edge tile:
    dma_gather bf16 source rows, build one-hot P[e,dst] on DVE with ONE
    tensor_scalar(iota == dloc[:,t]) (bf16 -> 4x DVE mode), accumulate
    M^T @ P into PSUM [f=128, dst=WIN] with bf16 matmuls.
  - Self-loops are identity-P matmuls fed straight from the SBUF hs block
    (never gathered, never compared).
  - Window epilogue: ONE tensor_tensor_reduce (pre = psum*dinv[dst],
    accum -> per-window sum) + ACT Square accum -> sumsq.  BN stats cross
    -core via AllGather of [128,2] partials + local reduce (cheaper than
    AllReduce).  y = relu(pre*s + t) as one big ACT op.
  - Output returned transposed bf16 [128, S] per core; host reassembles.

dma_gather indices are int16 (<32768); the G-row table is addressed
through two overlapping views lo=[0,32768) / hi=[G-32768,G); padding
slots point at dead all-zero rows.  b1/b2 are dropped: BN immediately
follows the +b and is invariant to per-feature constant shifts.
"""

import numpy as np

C = 8            # cores
D = 128          # feature dim
WIN = 128        # dst nodes per aggregation window (psum free dim)
GROUP_W = 5      # windows resident in PSUM at once
CH = 64          # max gather tiles (of 128 rows) per dma_gather call
MMB = 4          # phase_mm blocks batched per PSUM tile
IDX_LIMIT = 32768

_cache = {}


def _bf16():
    import ml_dtypes
    return ml_dtypes.bfloat16


def _plan(edge_index, N):
    """Host-side graph preprocessing -> per-core arrays + static structure."""
    src = np.asarray(edge_index[0], dtype=np.int64)
    dst = np.asarray(edge_index[1], dtype=np.int64)
    E = src.shape[0]

    Nr = -(-N // C)                      # real nodes per core
    S = -(-Nr // WIN) * WIN              # padded slots per core
    NW = S // WIN
    HN = S // 128
    G = C * S                            # table rows
    assert G > IDX_LIMIT and S - Nr >= 1, (G, S, Nr)

    deg = np.bincount(dst, minlength=N).astype(np.int64) + 1
    order = np.argsort(-deg, kind="stable")        # rank -> old id
    ranks = np.arange(N, dtype=np.int64)
    g_of_old = np.empty(N, dtype=np.int64)
    g_of_old[order] = (ranks % C) * S + ranks // C

    dinv = np.zeros(G, dtype=np.float64)
    dinv[g_of_old] = deg.astype(np.float64) ** -0.5

    gs = g_of_old[src]
    gd = g_of_old[dst]
    core_e = gd // S
    w_e = (gd % S) // WIN
    dl_e = (gd % S) % WIN
    view_e = (gs >= IDX_LIMIT).astype(np.int64)
    idx_e = np.where(view_e == 0, gs, gs - (G - IDX_LIMIT))
    assert idx_e.max() < IDX_LIMIT and idx_e.min() >= 0

    pad_idx = (S - 1, IDX_LIMIT - 1)     # core0 dead row (lo) / core7 dead (hi)

    counts = np.zeros((C, NW, 2), dtype=np.int64)
    np.add.at(counts, (core_e, w_e, view_e), 1)
    K = -(-counts.max(axis=0) // 128)             # [NW, 2] unified tile counts

    groups = [list(range(g0, min(g0 + GROUP_W, NW)))
              for g0 in range(0, NW, GROUP_W)]

    # global tile order: per group, [lo tiles (w asc)] then [hi tiles]
    tile_w, tile_v = [], []
    for grp in groups:
        for v in range(2):
            for w in grp:
                tile_w += [w] * int(K[w, v])
                tile_v += [v] * int(K[w, v])
    tile_w = np.array(tile_w, dtype=np.int64)
    tile_v = np.array(tile_v, dtype=np.int64)
    TOTK = len(tile_w)
    TOTS = TOTK * 128

    # chunks: runs of one view within a group, <= CH tiles.  The first
    # chunks of the layer are kept small so the gather->matmul pipeline
    # fills quickly after the table AllGather lands.
    chunks = []   # (tile0, ntiles, view)
    t = 0
    while t < TOTK:
        v = tile_v[t]
        cap = 8 if not chunks else (24 if len(chunks) == 1 else CH)
        r = t
        while r < TOTK and tile_v[r] == v and r - t < cap:
            r += 1
        chunks.append((int(t), int(r - t), int(v)))
        t = r
    first_tile, last_tile = {}, {}
    for t in range(TOTK):
        w = int(tile_w[t])
        first_tile.setdefault(w, t)
        last_tile[w] = t

    # slot base of each (w, v) segment in the global slot order
    tile_base = np.zeros((NW, 2), dtype=np.int64)
    b = 0
    for grp in groups:
        for v in range(2):
            for w in grp:
                tile_base[w, v] = b
                b += int(K[w, v]) * 128
    assert b == TOTS
    slot_view = np.empty(TOTS, dtype=np.int64)
    for w in range(NW):
        for v in range(2):
            s0 = int(tile_base[w, v]); n = int(K[w, v]) * 128
            slot_view[s0:s0 + n] = v

    ord_e = np.lexsort((dl_e, view_e, w_e, core_e))
    src_s = idx_e[ord_e]; core_s = core_e[ord_e]
    w_s = w_e[ord_e]; v_s = view_e[ord_e]; dl_s = dl_e[ord_e]

    idx_all = np.empty((C, TOTS), dtype=np.int16)
    dst_all = np.zeros((C, TOTS), dtype=np.float32)
    for c in range(C):
        m = core_s == c
        iw, iv, ii, idl = w_s[m], v_s[m], src_s[m], dl_s[m]
        arr_i = np.full(TOTS, -1, dtype=np.int64)
        arr_d = np.zeros(TOTS, dtype=np.int64)
        if len(iw):
            seg_id = iw * 2 + iv
            change = np.r_[True, np.diff(seg_id) != 0]
            seg_start = np.maximum.accumulate(
                np.where(change, np.arange(len(seg_id)), 0))
            within = np.arange(len(seg_id)) - seg_start
            flat = tile_base[iw, iv] + within
            arr_i[flat] = ii
            arr_d[flat] = idl
        padm = arr_i == -1
        arr_i[padm] = np.where(slot_view[padm] == 0, pad_idx[0], pad_idx[1])
        idx_all[c] = arr_i.astype(np.int16)
        dst_all[c] = arr_d.astype(np.float32)

    # dma_gather idx wrap: [128, TOTS/16] int16, 16-row pattern tiled x8
    idx_wrap = np.empty((C, 128, TOTS // 16), dtype=np.int16)
    for c in range(C):
        idx_wrap[c] = np.tile(idx_all[c].reshape(-1, 16).T, (8, 1))
    # dloc tile layout [128, TOTK]: [p, t] = dst_local of slot t*128+p
    dloc = dst_all.reshape(C, TOTK, 128).transpose(0, 2, 1).copy()

    bf16 = _bf16()
    dinv_f = dinv.astype(np.float32).reshape(C, S)
    dinvT = np.broadcast_to(
        dinv_f[:, None, :], (C, 128, S)).astype(bf16)
    dcols = dinv_f.reshape(C, HN, 128).transpose(0, 2, 1).astype(np.float32).copy()

    return dict(
        N=N, E=E, S=S, NW=NW, HN=HN, G=G, TOTK=TOTK, TOTS=TOTS,
        K=K, groups=groups, chunks=chunks, tile_w=tile_w, tile_v=tile_v,
        first_tile=first_tile, last_tile=last_tile,
        g_of_old=g_of_old, idx_wrap=idx_wrap, dloc=dloc,
        dinvT=dinvT, dcols=dcols, dinv_g=dinv.astype(np.float32),
    )


def _build(tc, outs, ins, plan):
    """Emit the Tile program. ins/outs: dicts of DRAM APs."""
    import contextlib

    import concourse.mybir as mybir

    nc = tc.nc
    S, NW, HN, G, TOTK = plan["S"], plan["NW"], plan["HN"], plan["G"], plan["TOTK"]
    N = plan["N"]
    K = plan["K"]
    f32 = mybir.dt.float32
    bf16 = mybir.dt.bfloat16
    i16 = mybir.dt.int16
    AF = mybir.ActivationFunctionType
    OP = mybir.AluOpType
    rg = [list(range(C))]

    hs_dram = [nc.dram_tensor(f"hs{l}", [S, D], bf16) for l in range(2)]
    table = [nc.dram_tensor(f"table{l}", [G, D], bf16, addr_space="Shared")
             for l in range(2)]
    bnin = [nc.dram_tensor(f"bnin{l}", [128, 2], f32) for l in range(2)]
    bnag = [nc.dram_tensor(f"bnag{l}", [C * 128, 2], f32, addr_space="Shared")
            for l in range(2)]

    ctx = contextlib.ExitStack()
    with ctx:
        persist = ctx.enter_context(tc.tile_pool(name="persist", bufs=1))
        gpool = ctx.enter_context(tc.tile_pool(name="gather", bufs=3))
        ppool = ctx.enter_context(tc.tile_pool(name="ptiles", bufs=48))
        spool = ctx.enter_context(tc.tile_pool(name="scratch", bufs=4))
        psagg = ctx.enter_context(tc.tile_pool(name="psagg", bufs=GROUP_W + 1,
                                               space="PSUM"))
        psmm = ctx.enter_context(tc.tile_pool(name="psmm", bufs=2, space="PSUM"))

        aT = persist.tile([128, S], bf16, tag="aT")      # activations [f, node]
        apre = persist.tile([128, S], bf16, tag="apre")  # pre-BN [f, node]
        hsAll = persist.tile([128, S], bf16, tag="hsAll")  # hs rows [node, f]
        dinvT = persist.tile([128, S], bf16, tag="dinvT")
        dloc = persist.tile([128, TOTK], f32, tag="dloc")
        idxs = persist.tile([128, plan["TOTS"] // 16], i16, tag="idxs")
        iota = persist.tile([128, WIN], bf16, tag="iota")
        ISW = (WIN // 128) * WIN
        Isel = persist.tile([128, ISW], bf16, tag="Isel")  # [I_0 | I_1 ...]
        Wt = [persist.tile([128, D], bf16, tag=f"W{l}", name=f"Wt{l}")
              for l in range(2)]
        gbs = persist.tile([128, 4], f32, tag="gbs")
        stats = persist.tile([128, 2 * NW], f32, tag="stats")
        bnsb = [persist.tile([128, 16], f32, tag=f"bnsb{l}", name=f"bnsb{l}")
                for l in range(2)]
        svec = [persist.tile([128, 1], f32, tag=f"svec{l}", name=f"svec{l}")
                for l in range(2)]
        tvec = [persist.tile([128, 1], f32, tag=f"tvec{l}", name=f"tvec{l}")
                for l in range(2)]

        # critical-path inputs first; the bulky agg-only inputs are issued
        # after phase_mm(0) so they overlap it on the DMA engines.
        nc.sync.dma_start(out=Wt[0][:], in_=ins["W1"][:, :])
        for j in range(4):
            csl = slice(j * (S // 4), (j + 1) * (S // 4))
            nc.sync.dma_start(out=aT[:, csl], in_=ins["xT"][:, csl])

        def load_aux():
            nc.sync.dma_start(out=dinvT[:], in_=ins["dinvT"][:, :])
            nc.sync.dma_start(out=dloc[:], in_=ins["dloc"][:, :])
            nc.sync.dma_start(out=idxs[:], in_=ins["idxw"][:, :])
            nc.sync.dma_start(out=iota[:], in_=ins["iota"][:, :])
            nc.sync.dma_start(out=Isel[:], in_=ins["isel"][:, :])
            nc.sync.dma_start(out=gbs[:], in_=ins["gb"][:, :])
            nc.sync.dma_start(out=Wt[1][:], in_=ins["W2"][:, :])

        def phase_mm(l, src_tile):
            """hsAll[node,f] = (src^T @ W); src is already scaled by dinv.

            MMB matmuls share one PSUM tile so the PSUM->SBUF copy and
            DRAM store are batched."""
            for h0 in range(0, HN, MMB):
                nb = min(MMB, HN - h0)
                psA = psmm.tile([128, MMB * D], f32, tag="psA")
                for k in range(nb):
                    sl = slice((h0 + k) * 128, (h0 + k + 1) * 128)
                    nc.tensor.matmul(out=psA[:, k * D:(k + 1) * D],
                                     lhsT=src_tile[:, sl], rhs=Wt[l][:],
                                     start=True, stop=True)
                bsl = slice(h0 * 128, (h0 + nb) * 128)
                nc.scalar.activation(out=hsAll[:, bsl], in_=psA[:, :nb * D],
                                     func=AF.Copy)
                nc.sync.dma_start(
                    out=hs_dram[l][h0 * 128:(h0 + nb) * 128, :]
                    .rearrange("(b p) f -> p b f", p=128),
                    in_=hsAll[:, bsl].rearrange("p (b f) -> p b f", f=D))

        def win_epilogue(l, w, psw_w):
            wsl = slice(w * WIN, (w + 1) * WIN)
            nc.vector.tensor_tensor(out=apre[:, wsl], in0=psw_w[:],
                                    in1=dinvT[:, wsl], op=OP.mult)
            sc = spool.tile([128, WIN], bf16, tag="sc")
            nc.scalar.activation(out=sc[:], in_=apre[:, wsl], func=AF.Copy,
                                 accum_out=stats[:, w:w + 1])
            sq = spool.tile([128, WIN], bf16, tag="sq")
            nc.scalar.activation(out=sq[:], in_=apre[:, wsl], func=AF.Square,
                                 accum_out=stats[:, NW + w:NW + w + 1])

        def phase_agg(l):
            lo_ap = table[l][0:IDX_LIMIT, :]
            hi_ap = table[l][G - IDX_LIMIT:G, :]
            psw = {}
            chunk_i = 0
            chunks = plan["chunks"]
            for grp in plan["groups"]:
                # self-loop tiles: identity P from the SBUF hs block
                for w in grp:
                    psw[w] = psagg.tile([128, WIN], f32, tag="psw",
                                        name=f"psw{l}_{w}")
                    nedge = int(K[w, 0] + K[w, 1])
                    for half in range(WIN // 128):
                        blk = slice((w * (WIN // 128) + half) * 128,
                                    (w * (WIN // 128) + half + 1) * 128)
                        nc.tensor.matmul(
                            out=psw[w][:],
                            lhsT=hsAll[:, blk],
                            rhs=Isel[:, half * WIN:(half + 1) * WIN],
                            start=(half == 0),
                            stop=(half == WIN // 128 - 1 and nedge == 0))
                    if nedge == 0:
                        win_epilogue(l, w, psw.pop(w))
                # edge tiles of this group, lo chunks then hi chunks
                grp_tiles = sum(int(K[w, v]) for w in grp for v in range(2))
                done = 0
                while done < grp_tiles:
                    (t0, nt, v) = chunks[chunk_i]
                    chunk_i += 1
                    done += nt
                    n_idx = nt * 128
                    gb = gpool.tile([128, CH * 128], bf16, tag="gbuf")
                    out3d = gb[:, :n_idx].rearrange("p (k f) -> p k f", f=D)
                    nc.gpsimd.dma_gather(
                        out_ap=out3d,
                        in_ap=(lo_ap if v == 0 else hi_ap),
                        idxs_ap=idxs[:, t0 * 8:(t0 + nt) * 8],
                        num_idxs=n_idx, num_idxs_reg=n_idx, elem_size=D,
                        single_packet=False,
                    )
                    for k in range(nt):
                        t = t0 + k
                        w = int(plan["tile_w"][t])
                        P = ppool.tile([128, WIN], bf16, tag="P")
                        nc.vector.tensor_scalar(
                            out=P[:], in0=iota[:], scalar1=dloc[:, t:t + 1],
                            scalar2=None, op0=OP.is_equal)
                        nc.tensor.matmul(
                            out=psw[w][:],
                            lhsT=gb[:, k * 128:(k + 1) * 128],
                            rhs=P[:],
                            start=False,
                            stop=(plan["last_tile"][w] == t))
                        if plan["last_tile"][w] == t:
                            win_epilogue(l, w, psw.pop(w))
            assert chunk_i == len(chunks) and not psw

        def phase_bn(l, dst_tile):
            ssum = spool.tile([128, 1], f32, tag="bns")
            ssq = spool.tile([128, 1], f32, tag="bnq")
            nc.vector.tensor_reduce(out=ssum[:], in_=stats[:, 0:NW],
                                    axis=mybir.AxisListType.X, op=OP.add)
            nc.vector.tensor_reduce(out=ssq[:], in_=stats[:, NW:2 * NW],
                                    axis=mybir.AxisListType.X, op=OP.add)
            pk = spool.tile([128, 2], f32, tag="bnpack")
            nc.vector.tensor_copy(out=pk[:, 0:1], in_=ssum[:])
            nc.vector.tensor_copy(out=pk[:, 1:2], in_=ssq[:])
            nc.sync.dma_start(out=bnin[l][:, :], in_=pk[:])
            nc.gpsimd.collective_compute(
                "AllGather", OP.bypass, replica_groups=rg,
                ins=[bnin[l].ap()], outs=[bnag[l].ap()])
            # bnag rows (r*128+p, k) -> sbuf [p, k, r]
            nc.sync.dma_start(
                out=bnsb[l][:].rearrange("p (k r) -> p k r", r=C),
                in_=bnag[l][:, :].rearrange("(r p) k -> p k r", p=128))
            st = bnsb[l]
            tsum = spool.tile([128, 1], f32, tag="tsum")
            tsq = spool.tile([128, 1], f32, tag="tsq")
            nc.vector.tensor_reduce(out=tsum[:], in_=st[:, 0:C],
                                    axis=mybir.AxisListType.X, op=OP.add)
            nc.vector.tensor_reduce(out=tsq[:], in_=st[:, C:2 * C],
                                    axis=mybir.AxisListType.X, op=OP.add)
            mean = spool.tile([128, 1], f32, tag="mean")
            var = spool.tile([128, 1], f32, tag="var")
            nc.vector.tensor_scalar(out=mean[:], in0=tsum[:],
                                    scalar1=1.0 / N, scalar2=None, op0=OP.mult)
            nc.vector.tensor_scalar(out=var[:], in0=tsq[:],
                                    scalar1=1.0 / N, scalar2=None, op0=OP.mult)
            m2 = spool.tile([128, 1], f32, tag="m2")
            nc.vector.tensor_tensor(out=m2[:], in0=mean[:], in1=mean[:],
                                    op=OP.mult)
            nc.vector.tensor_sub(out=var[:], in0=var[:], in1=m2[:])
            nc.vector.tensor_scalar(out=var[:], in0=var[:], scalar1=1e-5,
                                    scalar2=None, op0=OP.add)
            sd = spool.tile([128, 1], f32, tag="sd")
            nc.scalar.sqrt(out=sd[:], in_=var[:])
            rsd = spool.tile([128, 1], f32, tag="rsd")
            nc.vector.reciprocal(out=rsd[:], in_=sd[:])
            nc.vector.tensor_tensor(out=svec[l][:], in0=gbs[:, 2 * l:2 * l + 1],
                                    in1=rsd[:], op=OP.mult)
            ms = spool.tile([128, 1], f32, tag="ms")
            nc.vector.tensor_tensor(out=ms[:], in0=mean[:], in1=svec[l][:],
                                    op=OP.mult)
            nc.vector.tensor_sub(out=tvec[l][:], in0=gbs[:, 2 * l + 1:2 * l + 2],
                                 in1=ms[:])
            # relu in chunks so downstream per-block consumers can start
            # before the whole row is done; for layer 0 each chunk is also
            # immediately rescaled by dinv into apre for phase_mm(1).
            bounds = list(range(0, S, 2048)) + [S]
            for j in range(len(bounds) - 1):
                csl = slice(bounds[j], bounds[j + 1])
                nc.scalar.activation(out=dst_tile[:, csl], in_=apre[:, csl],
                                     func=AF.Relu, bias=tvec[l][:],
                                     scale=svec[l][:])
                if l == 0:
                    nc.vector.tensor_tensor(out=apre[:, csl],
                                            in0=dst_tile[:, csl],
                                            in1=dinvT[:, csl], op=OP.mult)

        # layer 1
        phase_mm(0, aT)
        load_aux()
        nc.gpsimd.collective_compute(
            "AllGather", OP.bypass, replica_groups=rg,
            ins=[hs_dram[0].ap()], outs=[table[0].ap()])
        phase_agg(0)
        phase_bn(0, aT)                # aT := y1, apre := y1*dinv
        # layer 2
        phase_mm(1, apre)
        nc.gpsimd.collective_compute(
            "AllGather", OP.bypass, replica_groups=rg,
            ins=[hs_dram[1].ap()], outs=[table[1].ap()])
        phase_agg(1)
        phase_bn(1, aT)                # aT := y2
        q = S // 2
        for j in range(2):
            csl = slice(j * q, (j + 1) * q)
            nc.sync.dma_start(out=outs["outT"][:, csl], in_=aT[:, csl])


def _make_inputs(plan, x, W1, W2, gamma1, beta1, gamma2, beta2):
    bf16 = _bf16()
    S, G = plan["S"], plan["G"]
    xg = np.zeros((G, D), dtype=np.float32)
    xg[plan["g_of_old"]] = np.asarray(x, dtype=np.float32)
    xg *= plan["dinv_g"][:, None]          # phase_mm expects dinv-scaled input
    xg = xg.astype(bf16)
    iota = np.tile(np.arange(WIN, dtype=np.float32), (128, 1)).astype(bf16)
    # [I_lo | I_hi]: I_half[p, d] = (d == half*128 + p)
    p = np.arange(128)[:, None]
    d = np.arange(WIN)[None, :]
    isel = np.concatenate(
        [(d == h * 128 + p).astype(np.float32) for h in range(WIN // 128)],
        axis=1).astype(bf16)
    gb = np.stack([np.asarray(gamma1, np.float32), np.asarray(beta1, np.float32),
                   np.asarray(gamma2, np.float32), np.asarray(beta2, np.float32)],
                  axis=1).copy()
    in_maps = []
    for c in range(C):
        in_maps.append({
            "xT": np.ascontiguousarray(xg[c * S:(c + 1) * S].T),
            "idxw": plan["idx_wrap"][c],
            "dloc": plan["dloc"][c],
            "dinvT": np.ascontiguousarray(plan["dinvT"][c]),
            "W1": np.asarray(W1, np.float32).astype(bf16),
            "W2": np.asarray(W2, np.float32).astype(bf16),
            "gb": gb, "iota": iota, "isel": isel,
        })
    return in_maps


def _declare_io(nc, plan):
    import concourse.mybir as mybir
    f32, i16, bf16 = mybir.dt.float32, mybir.dt.int16, mybir.dt.bfloat16
    S, HN, TOTK, TOTS = plan["S"], plan["HN"], plan["TOTK"], plan["TOTS"]
    ins = {
        "xT": nc.dram_tensor("xT", [128, S], bf16, kind="ExternalInput"),
        "idxw": nc.dram_tensor("idxw", [128, TOTS // 16], i16,
                               kind="ExternalInput"),
        "dloc": nc.dram_tensor("dloc", [128, TOTK], f32, kind="ExternalInput"),
        "dinvT": nc.dram_tensor("dinvT", [128, S], bf16, kind="ExternalInput"),
        "W1": nc.dram_tensor("W1", [128, D], bf16, kind="ExternalInput"),
        "W2": nc.dram_tensor("W2", [128, D], bf16, kind="ExternalInput"),
        "gb": nc.dram_tensor("gb", [128, 4], f32, kind="ExternalInput"),
        "iota": nc.dram_tensor("iota", [128, WIN], bf16, kind="ExternalInput"),
        "isel": nc.dram_tensor("isel", [128, (WIN // 128) * WIN], bf16,
                               kind="ExternalInput"),
    }
    outs = {"outT": nc.dram_tensor("outT", [128, S], bf16,
                                   kind="ExternalOutput")}
    return ins, outs


def _compile(plan, trace_sim=False):
    from concourse import bacc, tile

    nc = bacc.Bacc("TRN2", target_bir_lowering=False, debug=False, num_devices=C)
    ins, outs = _declare_io(nc, plan)
    with tile.TileContext(nc, trace_sim=trace_sim) as tc:
        _build(tc, outs, ins, plan)
    nc.compile()
    return nc


def _run(x, edge_index, W1, b1, gamma1, beta1, W2, b2, gamma2, beta2,
         trace=False):
    from concourse.bass_utils import run_bass_kernel_spmd

    N = x.shape[0]
    plan = _plan(np.asarray(edge_index), N)
    key = (N, plan["E"], plan["TOTK"], tuple(int(k) for k in plan["K"].ravel()))
    if key not in _cache:
        _cache.clear()
        _cache[key] = _compile(plan)
    nc = _cache[key]
    in_maps = _make_inputs(plan, x, W1, W2, gamma1, beta1, gamma2, beta2)
    res = run_bass_kernel_spmd(nc, in_maps, core_ids=list(range(C)), trace=trace)
    S, G = plan["S"], plan["G"]
    aT_full = np.concatenate(
        [np.asarray(res.results[c]["outT"], dtype=np.float32) for c in range(C)],
        axis=1)
    assert aT_full.shape == (128, G)
    y = np.ascontiguousarray(aT_full.T[plan["g_of_old"]], dtype=np.float32)
    return y, res


def kernel(**inputs):
    y, _ = _run(**inputs)
    return y


# revision 40
# speedup vs baseline: 1.5907x; 1.3254x over previous
"""2-layer GCN (PyG GCNConv + BN + ReLU) on 8 Trainium2 NeuronCores.

Strategy (node sharding, fp16 data path; ~2.7x the session-start kernel):
  - Nodes dealt round-robin by in-degree to 8 cores; within a core they
    are snake-dealt across the 49 dst windows so per-window edge counts
    are near-uniform (minimal tile padding).
  - Per layer: hs = ((a*dinv) @ W) via MMB-batched matmuls into one PSUM
    tile + one ACT copy to an SBUF [node,f] fp16 block + one DRAM store;
    AllGather -> full fp16 table [G,128] in every core's DRAM.
  - Edges partitioned by destination core into 128-dst windows processed
    in PSUM-resident groups of GROUP_W.  Per 128-edge tile: dma_gather
    fp16 source rows (single_packet=False, 2 SWDGE queues), build one-hot
    P[e,dst] with ONE tensor_scalar(iota == dloc[:,t]) (fp16 -> 4x DVE
    mode; prebuilt ahead into a 96-deep pool to fill stall windows),
    accumulate M^T @ P into PSUM [f=128, dst=128] with fp16 matmuls.
  - Self-loops are identity-P matmuls fed straight from the SBUF hs block
    (never gathered, never compared).
  - Window epilogue: ACT copy PSUM->SBUF, DVE mult by dinv[dst] (2x),
    ACT Copy/Square accum_out -> per-window sum/sumsq.  BN stats cross-
    core via AllGather of [128,2] partials + local reduce (cheaper than
    AllReduce).  y = relu(pre*s + t) in chunks overlapping the next
    consumer (phase_mm prescale or the output DMA).

dma_gather indices are int16 (<32768); the G-row table is addressed
through two overlapping views lo=[0,32768) / hi=[G-32768,G).  Views are
assigned by SOURCE CORE (cores 0-2 lo, 5-7 hi; 3-4 chosen per (dst core,
window) to steer both counts just under 128-tile boundaries).  Padding
slots point at dead all-zero rows.  b1/b2 are dropped: BN immediately
follows the +b and is invariant to per-feature constant shifts.
"""

import numpy as np

C = 8            # cores
D = 128          # feature dim
WIN = 128        # dst nodes per aggregation window (psum free dim)
GROUP_W = 5      # windows resident in PSUM at once
CH = 64          # max gather tiles (of 128 rows) per dma_gather call
MMB = 4          # phase_mm blocks batched per PSUM tile
IDX_LIMIT = 32768

_cache = {}


def _bf16():
    return np.float16


def _plan(edge_index, N):
    """Host-side graph preprocessing -> per-core arrays + static structure."""
    src = np.asarray(edge_index[0], dtype=np.int64)
    dst = np.asarray(edge_index[1], dtype=np.int64)
    E = src.shape[0]

    Nr = -(-N // C)                      # real nodes per core
    S = -(-Nr // WIN) * WIN              # padded slots per core
    NW = S // WIN
    HN = S // 128
    G = C * S                            # table rows
    assert G > IDX_LIMIT and S - Nr >= 1, (G, S, Nr)

    deg = np.bincount(dst, minlength=N).astype(np.int64) + 1
    order = np.argsort(-deg, kind="stable")        # rank -> old id
    ranks = np.arange(N, dtype=np.int64)
    core_of_old = np.empty(N, dtype=np.int64)
    core_of_old[order] = ranks % C
    # Edge views are set by SOURCE CORE (placement-independent), so the
    # per-node lo/hi in-edge counts are known before slot placement.
    lo_in = np.bincount(dst[core_of_old[src] < 5], minlength=N)
    # Within each core, spread nodes over ALL NW windows (so per-window
    # counts sit just under tile boundaries) and snake-deal by lo-degree
    # so per-window lo edge counts are near-equal -> minimal tile padding.
    g_of_old = np.empty(N, dtype=np.int64)
    for c in range(C):
        nodes_c = order[core_of_old[order] == c]
        nodes_c = nodes_c[np.argsort(-lo_in[nodes_c], kind="stable")]
        n = len(nodes_c)
        i = np.arange(n)
        rnd, pos = i // NW, i % NW
        # snake, except the final partial round fills ascending so the
        # highest slots (the dead pad rows, incl. S-1 / G-1) stay empty
        w = np.where((rnd % 2 == 0) | (rnd == (n - 1) // NW),
                     pos, NW - 1 - pos)
        g_of_old[nodes_c] = c * S + w * WIN + rnd
    assert np.all(g_of_old % S != S - 1), "pad slot S-1 must stay dead"

    dinv = np.zeros(G, dtype=np.float64)
    dinv[g_of_old] = deg.astype(np.float64) ** -0.5

    gs = g_of_old[src]
    gd = g_of_old[dst]
    core_e = gd // S
    w_e = (gd % S) // WIN
    dl_e = (gd % S) % WIN
    # Views by SOURCE CORE (placement-independent): cores 0-4 fit the lo
    # window [0, 32768); cores 3-7 fit the hi window [G-32768, G).  Edges
    # sourced from cores 3-4 are FLEXIBLE: assign them per (dst core,
    # window) to steer lo/hi counts just under tile boundaries.
    src_core = gs // S
    flex = (src_core == 3) | (src_core == 4)
    view_e = (src_core >= 5).astype(np.int64)
    cwkey = core_e * NW + w_e
    nA = np.bincount(cwkey[(~flex) & (view_e == 0)], minlength=C * NW)
    nB = np.bincount(cwkey[(~flex) & (view_e == 1)], minlength=C * NW)
    nF = np.bincount(cwkey[flex], minlength=C * NW)
    # per WINDOW pick one lo-tile target klo shared by all cores (the
    # unified K takes a max over cores, so cores must agree on the split),
    # then each core fills lo up to klo*128 with its flexible edges.
    A2 = nA.reshape(C, NW)
    B2 = nB.reshape(C, NW)
    F2 = nF.reshape(C, NW)
    f_pick = np.zeros((C, NW), dtype=np.int64)
    for w in range(NW):
        kmin = int(-(-A2[:, w].max() // 128))
        best = (1 << 30, kmin)
        for klo in range(kmin, kmin + 3):
            f = np.minimum(F2[:, w], klo * 128 - A2[:, w])
            khi = int(-(-(B2[:, w] + F2[:, w] - f).max() // 128))
            if klo + khi < best[0]:
                best = (klo + khi, klo)
        klo = best[1]
        f_pick[:, w] = np.clip(klo * 128 - A2[:, w], 0, F2[:, w])
    f_pick = f_pick.reshape(C * NW)
    # rank flex edges within their (c,w) bucket; first f_pick go to lo
    mflex = flex
    of = np.lexsort((np.arange(E)[mflex], cwkey[mflex]))
    sf = cwkey[mflex][of]
    rs = np.r_[True, np.diff(sf) != 0]
    ir = np.maximum.accumulate(np.where(rs, np.arange(len(sf)), 0))
    wr = np.arange(len(sf)) - ir
    fr = np.empty(len(sf), dtype=np.int64)
    fr[of] = wr
    flex_to_lo = fr < f_pick[cwkey[mflex]]
    view_e[mflex] = np.where(flex_to_lo, 0, 1)
    idx_e = np.where(view_e == 0, gs, gs - (G - IDX_LIMIT))
    assert idx_e.max() < IDX_LIMIT and idx_e.min() >= 0
    assert np.all(np.where(view_e == 1, gs >= G - IDX_LIMIT, gs < IDX_LIMIT))

    pad_idx = (S - 1, IDX_LIMIT - 1)     # core0 dead row (lo) / core7 dead (hi)

    counts = np.zeros((C, NW, 2), dtype=np.int64)
    np.add.at(counts, (core_e, w_e, view_e), 1)
    K = -(-counts.max(axis=0) // 128)             # [NW, 2] unified tile counts

    groups = [list(range(g0, min(g0 + GROUP_W, NW)))
              for g0 in range(0, NW, GROUP_W)]

    # global tile order: per group, [lo tiles (w asc)] then [hi tiles]
    tile_w, tile_v = [], []
    for grp in groups:
        for v in range(2):
            for w in grp:
                tile_w += [w] * int(K[w, v])
                tile_v += [v] * int(K[w, v])
    tile_w = np.array(tile_w, dtype=np.int64)
    tile_v = np.array(tile_v, dtype=np.int64)
    TOTK = len(tile_w)
    TOTS = TOTK * 128

    # chunks: runs of one view within a group, <= CH tiles.  The first
    # chunks of the layer are kept small so the gather->matmul pipeline
    # fills quickly after the table AllGather lands.
    chunks = []   # (tile0, ntiles, view)
    t = 0
    while t < TOTK:
        v = tile_v[t]
        cap = 8 if not chunks else (24 if len(chunks) == 1 else CH)
        if TOTK - t <= 32:
            cap = min(cap, 12)       # small drain chunks at the layer tail
        r = t
        while r < TOTK and tile_v[r] == v and r - t < cap:
            r += 1
        chunks.append((int(t), int(r - t), int(v)))
        t = r
    first_tile, last_tile = {}, {}
    for t in range(TOTK):
        w = int(tile_w[t])
        first_tile.setdefault(w, t)
        last_tile[w] = t

    # slot base of each (w, v) segment in the global slot order
    tile_base = np.zeros((NW, 2), dtype=np.int64)
    b = 0
    for grp in groups:
        for v in range(2):
            for w in grp:
                tile_base[w, v] = b
                b += int(K[w, v]) * 128
    assert b == TOTS
    slot_view = np.empty(TOTS, dtype=np.int64)
    for w in range(NW):
        for v in range(2):
            s0 = int(tile_base[w, v]); n = int(K[w, v]) * 128
            slot_view[s0:s0 + n] = v

    ord_e = np.lexsort((dl_e, view_e, w_e, core_e))
    src_s = idx_e[ord_e]; core_s = core_e[ord_e]
    w_s = w_e[ord_e]; v_s = view_e[ord_e]; dl_s = dl_e[ord_e]

    idx_all = np.empty((C, TOTS), dtype=np.int16)
    dst_all = np.zeros((C, TOTS), dtype=np.float32)
    for c in range(C):
        m = core_s == c
        iw, iv, ii, idl = w_s[m], v_s[m], src_s[m], dl_s[m]
        arr_i = np.full(TOTS, -1, dtype=np.int64)
        arr_d = np.zeros(TOTS, dtype=np.int64)
        if len(iw):
            seg_id = iw * 2 + iv
            change = np.r_[True, np.diff(seg_id) != 0]
            seg_start = np.maximum.accumulate(
                np.where(change, np.arange(len(seg_id)), 0))
            within = np.arange(len(seg_id)) - seg_start
            flat = tile_base[iw, iv] + within
            arr_i[flat] = ii
            arr_d[flat] = idl
        padm = arr_i == -1
        arr_i[padm] = np.where(slot_view[padm] == 0, pad_idx[0], pad_idx[1])
        idx_all[c] = arr_i.astype(np.int16)
        dst_all[c] = arr_d.astype(np.float32)

    # dma_gather idx wrap: [128, TOTS/16] int16, 16-row pattern tiled x8
    idx_wrap = np.empty((C, 128, TOTS // 16), dtype=np.int16)
    for c in range(C):
        idx_wrap[c] = np.tile(idx_all[c].reshape(-1, 16).T, (8, 1))
    # dloc tile layout [128, TOTK]: [p, t] = dst_local of slot t*128+p
    dloc = dst_all.reshape(C, TOTK, 128).transpose(0, 2, 1).copy()

    bf16 = _bf16()
    dinv_f = dinv.astype(np.float32).reshape(C, S)
    dinvT = np.broadcast_to(
        dinv_f[:, None, :], (C, 128, S)).astype(bf16)
    dcols = dinv_f.reshape(C, HN, 128).transpose(0, 2, 1).astype(np.float32).copy()

    return dict(
        N=N, E=E, S=S, NW=NW, HN=HN, G=G, TOTK=TOTK, TOTS=TOTS,
        K=K, groups=groups, chunks=chunks, tile_w=tile_w, tile_v=tile_v,
        first_tile=first_tile, last_tile=last_tile,
        g_of_old=g_of_old, idx_wrap=idx_wrap, dloc=dloc,
        dinvT=dinvT, dcols=dcols, dinv_g=dinv.astype(np.float32),
    )


def _build(tc, outs, ins, plan):
    """Emit the Tile program. ins/outs: dicts of DRAM APs."""
    import contextlib

    import concourse.mybir as mybir

    nc = tc.nc
    S, NW, HN, G, TOTK = plan["S"], plan["NW"], plan["HN"], plan["G"], plan["TOTK"]
    N = plan["N"]
    K = plan["K"]
    f32 = mybir.dt.float32
    bf16 = mybir.dt.float16
    i16 = mybir.dt.int16
    AF = mybir.ActivationFunctionType
    OP = mybir.AluOpType
    rg = [list(range(C))]

    hs_dram = [nc.dram_tensor(f"hs{l}", [S, D], bf16) for l in range(2)]
    table = [nc.dram_tensor(f"table{l}", [G, D], bf16, addr_space="Shared")
             for l in range(2)]
    bnin = [nc.dram_tensor(f"bnin{l}", [128, 2], f32) for l in range(2)]
    bnag = [nc.dram_tensor(f"bnag{l}", [C * 128, 2], f32, addr_space="Shared")
            for l in range(2)]

    ctx = contextlib.ExitStack()
    with ctx:
        persist = ctx.enter_context(tc.tile_pool(name="persist", bufs=1))
        gpool = ctx.enter_context(tc.tile_pool(name="gather", bufs=3))
        ppool = ctx.enter_context(tc.tile_pool(name="ptiles", bufs=96))
        spool = ctx.enter_context(tc.tile_pool(name="scratch", bufs=4))
        psagg = ctx.enter_context(tc.tile_pool(name="psagg", bufs=GROUP_W + 1,
                                               space="PSUM"))
        psmm = ctx.enter_context(tc.tile_pool(name="psmm", bufs=2, space="PSUM"))

        aT = persist.tile([128, S], bf16, tag="aT")      # activations [f, node]
        apre = persist.tile([128, S], bf16, tag="apre")  # pre-BN [f, node]
        hsAll = persist.tile([128, S], bf16, tag="hsAll")  # hs rows [node, f]
        dinvT = persist.tile([128, S], bf16, tag="dinvT")
        dloc = persist.tile([128, TOTK], f32, tag="dloc")
        idxs = persist.tile([128, plan["TOTS"] // 16], i16, tag="idxs")
        iota = persist.tile([128, WIN], bf16, tag="iota")
        ISW = (WIN // 128) * WIN
        Isel = persist.tile([128, ISW], bf16, tag="Isel")  # [I_0 | I_1 ...]
        Wt = [persist.tile([128, D], bf16, tag=f"W{l}", name=f"Wt{l}")
              for l in range(2)]
        gbs = persist.tile([128, 4], f32, tag="gbs")
        stats = persist.tile([128, 2 * NW], f32, tag="stats")
        bnsb = [persist.tile([128, 16], f32, tag=f"bnsb{l}", name=f"bnsb{l}")
                for l in range(2)]
        svec = [persist.tile([128, 1], f32, tag=f"svec{l}", name=f"svec{l}")
                for l in range(2)]
        tvec = [persist.tile([128, 1], f32, tag=f"tvec{l}", name=f"tvec{l}")
                for l in range(2)]

        # critical-path inputs first; the bulky agg-only inputs are issued
        # after phase_mm(0) so they overlap it on the DMA engines.
        nc.sync.dma_start(out=Wt[0][:], in_=ins["W1"][:, :])
        bounds = [0, 512, 1536, 3072, S]
        for j in range(len(bounds) - 1):
            csl = slice(bounds[j], bounds[j + 1])
            nc.sync.dma_start(out=aT[:, csl], in_=ins["xT"][:, csl])

        def load_aux():
            nc.scalar.dma_start(out=dinvT[:], in_=ins["dinvT"][:, :])
            nc.scalar.dma_start(out=dloc[:], in_=ins["dloc"][:, :])
            nc.scalar.dma_start(out=idxs[:], in_=ins["idxw"][:, :])
            nc.scalar.dma_start(out=iota[:], in_=ins["iota"][:, :])
            nc.scalar.dma_start(out=Isel[:], in_=ins["isel"][:, :])
            nc.scalar.dma_start(out=gbs[:], in_=ins["gb"][:, :])
            nc.scalar.dma_start(out=Wt[1][:], in_=ins["W2"][:, :])

        def phase_mm(l, src_tile):
            """hsAll[node,f] = (src^T @ W); src is already scaled by dinv.

            MMB matmuls share one PSUM tile so the PSUM->SBUF copy and
            DRAM store are batched."""
            for h0 in range(0, HN, MMB):
                nb = min(MMB, HN - h0)
                psA = psmm.tile([128, MMB * D], f32, tag="psA")
                for k in range(nb):
                    sl = slice((h0 + k) * 128, (h0 + k + 1) * 128)
                    nc.tensor.matmul(out=psA[:, k * D:(k + 1) * D],
                                     lhsT=src_tile[:, sl], rhs=Wt[l][:],
                                     start=True, stop=True)
                bsl = slice(h0 * 128, (h0 + nb) * 128)
                nc.scalar.activation(out=hsAll[:, bsl], in_=psA[:, :nb * D],
                                     func=AF.Copy)
                nc.sync.dma_start(
                    out=hs_dram[l][h0 * 128:(h0 + nb) * 128, :]
                    .rearrange("(b p) f -> p b f", p=128),
                    in_=hsAll[:, bsl].rearrange("p (b f) -> p b f", f=D))

        def win_epilogue(l, w, psw_w):
            # ACT copies PSUM->SBUF (frees the bank early, narrows to fp16)
            # so the DVE multiply runs in 2x mode on SBUF operands.
            wsl = slice(w * WIN, (w + 1) * WIN)
            sc = spool.tile([128, WIN], bf16, tag="sc")
            nc.scalar.activation(out=sc[:], in_=psw_w[:], func=AF.Copy)
            nc.vector.tensor_tensor(out=apre[:, wsl], in0=sc[:],
                                    in1=dinvT[:, wsl], op=OP.mult)
            su = spool.tile([128, WIN], bf16, tag="su")
            nc.scalar.activation(out=su[:], in_=apre[:, wsl], func=AF.Copy,
                                 accum_out=stats[:, w:w + 1])
            sq = spool.tile([128, WIN], bf16, tag="sq")
            nc.scalar.activation(out=sq[:], in_=apre[:, wsl], func=AF.Square,
                                 accum_out=stats[:, NW + w:NW + w + 1])

        prebuilt = {}

        def build_P(t):
            P = ppool.tile([128, WIN], bf16, tag="P")
            nc.vector.tensor_scalar(
                out=P[:], in0=iota[:], scalar1=dloc[:, t:t + 1],
                scalar2=None, op0=OP.is_equal)
            return P

        def prebuild(n):
            # one-hot P tiles depend only on static inputs; emitting them
            # early lets the DVE fill collective/BN stall windows.
            for t in range(min(n, TOTK)):
                prebuilt[t] = build_P(t)

        def phase_agg(l):
            lo_ap = table[l][0:IDX_LIMIT, :]
            hi_ap = table[l][G - IDX_LIMIT:G, :]
            psw = {}
            chunk_i = 0
            chunks = plan["chunks"]
            for grp in plan["groups"]:
                # self-loop tiles: identity P from the SBUF hs block
                for w in grp:
                    psw[w] = psagg.tile([128, WIN], f32, tag="psw",
                                        name=f"psw{l}_{w}")
                    nedge = int(K[w, 0] + K[w, 1])
                    for half in range(WIN // 128):
                        blk = slice((w * (WIN // 128) + half) * 128,
                                    (w * (WIN // 128) + half + 1) * 128)
                        nc.tensor.matmul(
                            out=psw[w][:],
                            lhsT=hsAll[:, blk],
                            rhs=Isel[:, half * WIN:(half + 1) * WIN],
                            start=(half == 0),
                            stop=(half == WIN // 128 - 1 and nedge == 0))
                    if nedge == 0:
                        win_epilogue(l, w, psw.pop(w))
                # edge tiles of this group, lo chunks then hi chunks
                grp_tiles = sum(int(K[w, v]) for w in grp for v in range(2))
                done = 0
                while done < grp_tiles:
                    (t0, nt, v) = chunks[chunk_i]
                    qn = chunk_i % 2
                    chunk_i += 1
                    done += nt
                    n_idx = nt * 128
                    gb = gpool.tile([128, CH * 128], bf16, tag="gbuf")
                    out3d = gb[:, :n_idx].rearrange("p (k f) -> p k f", f=D)
                    nc.gpsimd.dma_gather(
                        out_ap=out3d,
                        in_ap=(lo_ap if v == 0 else hi_ap),
                        idxs_ap=idxs[:, t0 * 8:(t0 + nt) * 8],
                        num_idxs=n_idx, num_idxs_reg=n_idx, elem_size=D,
                        single_packet=False, queue_num=qn,
                    )
                    for k in range(nt):
                        t = t0 + k
                        w = int(plan["tile_w"][t])
                        P = prebuilt.pop(t, None)
                        if P is None:
                            P = build_P(t)
                        nc.tensor.matmul(
                            out=psw[w][:],
                            lhsT=gb[:, k * 128:(k + 1) * 128],
                            rhs=P[:],
                            start=False,
                            stop=(plan["last_tile"][w] == t))
                        if plan["last_tile"][w] == t:
                            win_epilogue(l, w, psw.pop(w))
            assert chunk_i == len(chunks) and not psw

        def phase_bn(l, dst_tile):
            ssum = spool.tile([128, 1], f32, tag="bns")
            ssq = spool.tile([128, 1], f32, tag="bnq")
            nc.vector.tensor_reduce(out=ssum[:], in_=stats[:, 0:NW],
                                    axis=mybir.AxisListType.X, op=OP.add)
            nc.vector.tensor_reduce(out=ssq[:], in_=stats[:, NW:2 * NW],
                                    axis=mybir.AxisListType.X, op=OP.add)
            pk = spool.tile([128, 2], f32, tag="bnpack")
            nc.vector.tensor_copy(out=pk[:, 0:1], in_=ssum[:])
            nc.vector.tensor_copy(out=pk[:, 1:2], in_=ssq[:])
            nc.sync.dma_start(out=bnin[l][:, :], in_=pk[:])
            nc.gpsimd.collective_compute(
                "AllGather", OP.bypass, replica_groups=rg,
                ins=[bnin[l].ap()], outs=[bnag[l].ap()])
            # bnag rows (r*128+p, k) -> sbuf [p, k, r]
            nc.sync.dma_start(
                out=bnsb[l][:].rearrange("p (k r) -> p k r", r=C),
                in_=bnag[l][:, :].rearrange("(r p) k -> p k r", p=128))
            st = bnsb[l]
            tsum = spool.tile([128, 1], f32, tag="tsum")
            tsq = spool.tile([128, 1], f32, tag="tsq")
            nc.vector.tensor_reduce(out=tsum[:], in_=st[:, 0:C],
                                    axis=mybir.AxisListType.X, op=OP.add)
            nc.vector.tensor_reduce(out=tsq[:], in_=st[:, C:2 * C],
                                    axis=mybir.AxisListType.X, op=OP.add)
            mean = spool.tile([128, 1], f32, tag="mean")
            var = spool.tile([128, 1], f32, tag="var")
            nc.vector.tensor_scalar(out=mean[:], in0=tsum[:],
                                    scalar1=1.0 / N, scalar2=None, op0=OP.mult)
            nc.vector.tensor_scalar(out=var[:], in0=tsq[:],
                                    scalar1=1.0 / N, scalar2=None, op0=OP.mult)
            m2 = spool.tile([128, 1], f32, tag="m2")
            nc.vector.tensor_tensor(out=m2[:], in0=mean[:], in1=mean[:],
                                    op=OP.mult)
            nc.vector.tensor_sub(out=var[:], in0=var[:], in1=m2[:])
            nc.vector.tensor_scalar(out=var[:], in0=var[:], scalar1=1e-5,
                                    scalar2=None, op0=OP.add)
            sd = spool.tile([128, 1], f32, tag="sd")
            nc.scalar.sqrt(out=sd[:], in_=var[:])
            rsd = spool.tile([128, 1], f32, tag="rsd")
            nc.vector.reciprocal(out=rsd[:], in_=sd[:])
            nc.vector.tensor_tensor(out=svec[l][:], in0=gbs[:, 2 * l:2 * l + 1],
                                    in1=rsd[:], op=OP.mult)
            ms = spool.tile([128, 1], f32, tag="ms")
            nc.vector.tensor_tensor(out=ms[:], in0=mean[:], in1=svec[l][:],
                                    op=OP.mult)
            nc.vector.tensor_sub(out=tvec[l][:], in0=gbs[:, 2 * l + 1:2 * l + 2],
                                 in1=ms[:])
            # relu in chunks so downstream per-block consumers can start
            # before the whole row is done; for layer 0 each chunk is also
            # immediately rescaled by dinv into apre for phase_mm(1).
            if l == 0:
                bounds = [0, 512, 1536, 3072, 4608, S]
            else:
                bounds = list(range(0, S, 896)) + [S]
            for j in range(len(bounds) - 1):
                csl = slice(bounds[j], bounds[j + 1])
                nc.scalar.activation(out=dst_tile[:, csl], in_=apre[:, csl],
                                     func=AF.Relu, bias=tvec[l][:],
                                     scale=svec[l][:])
                if l == 0:
                    nc.vector.tensor_tensor(out=apre[:, csl],
                                            in0=dst_tile[:, csl],
                                            in1=dinvT[:, csl], op=OP.mult)
                else:
                    nc.sync.dma_start(out=outs["outT"][:, csl],
                                      in_=dst_tile[:, csl])

        # layer 1
        phase_mm(0, aT)
        load_aux()
        prebuild(90)
        nc.gpsimd.collective_compute(
            "AllGather", OP.bypass, replica_groups=rg,
            ins=[hs_dram[0].ap()], outs=[table[0].ap()])
        phase_agg(0)
        prebuild(90)                   # layer 2 shares the same P tiles
        phase_bn(0, aT)                # aT := y1, apre := y1*dinv
        # layer 2
        phase_mm(1, apre)
        nc.gpsimd.collective_compute(
            "AllGather", OP.bypass, replica_groups=rg,
            ins=[hs_dram[1].ap()], outs=[table[1].ap()])
        phase_agg(1)
        phase_bn(1, aT)                # aT := y2 (outT DMAs inside)


def _make_inputs(plan, x, W1, W2, gamma1, beta1, gamma2, beta2):
    bf16 = _bf16()
    S, G = plan["S"], plan["G"]
    xg = np.zeros((G, D), dtype=np.float32)
    xg[plan["g_of_old"]] = np.asarray(x, dtype=np.float32)
    xg *= plan["dinv_g"][:, None]          # phase_mm expects dinv-scaled input
    xg = xg.astype(bf16)
    iota = np.tile(np.arange(WIN, dtype=np.float32), (128, 1)).astype(bf16)
    # [I_lo | I_hi]: I_half[p, d] = (d == half*128 + p)
    p = np.arange(128)[:, None]
    d = np.arange(WIN)[None, :]
    isel = np.concatenate(
        [(d == h * 128 + p).astype(np.float32) for h in range(WIN // 128)],
        axis=1).astype(bf16)
    gb = np.stack([np.asarray(gamma1, np.float32), np.asarray(beta1, np.float32),
                   np.asarray(gamma2, np.float32), np.asarray(beta2, np.float32)],
                  axis=1).copy()
    in_maps = []
    for c in range(C):
        in_maps.append({
            "xT": np.ascontiguousarray(xg[c * S:(c + 1) * S].T),
            "idxw": plan["idx_wrap"][c],
            "dloc": plan["dloc"][c],
            "dinvT": np.ascontiguousarray(plan["dinvT"][c]),
            "W1": np.asarray(W1, np.float32).astype(bf16),
            "W2": np.asarray(W2, np.float32).astype(bf16),
            "gb": gb, "iota": iota, "isel": isel,
        })
    return in_maps


def _declare_io(nc, plan):
    import concourse.mybir as mybir
    f32, i16, bf16 = mybir.dt.float32, mybir.dt.int16, mybir.dt.float16
    S, HN, TOTK, TOTS = plan["S"], plan["HN"], plan["TOTK"], plan["TOTS"]
    ins = {
        "xT": nc.dram_tensor("xT", [128, S], bf16, kind="ExternalInput"),
        "idxw": nc.dram_tensor("idxw", [128, TOTS // 16], i16,
                               kind="ExternalInput"),
        "dloc": nc.dram_tensor("dloc", [128, TOTK], f32, kind="ExternalInput"),
        "dinvT": nc.dram_tensor("dinvT", [128, S], bf16, kind="ExternalInput"),
        "W1": nc.dram_tensor("W1", [128, D], bf16, kind="ExternalInput"),
        "W2": nc.dram_tensor("W2", [128, D], bf16, kind="ExternalInput"),
        "gb": nc.dram_tensor("gb", [128, 4], f32, kind="ExternalInput"),
        "iota": nc.dram_tensor("iota", [128, WIN], bf16, kind="ExternalInput"),
        "isel": nc.dram_tensor("isel", [128, (WIN // 128) * WIN], bf16,
                               kind="ExternalInput"),
    }
    outs = {"outT": nc.dram_tensor("outT", [128, S], bf16,
                                   kind="ExternalOutput")}
    return ins, outs


def _compile(plan, trace_sim=False):
    from concourse import bacc, tile

    nc = bacc.Bacc("TRN2", target_bir_lowering=False, debug=False, num_devices=C,
                   num_swdge_queues=2, dynamic_dma_scratch_size=65536)
    ins, outs = _declare_io(nc, plan)
    with tile.TileContext(nc, trace_sim=trace_sim) as tc:
        _build(tc, outs, ins, plan)
    nc.compile()
    return nc


def _run(x, edge_index, W1, b1, gamma1, beta1, W2, b2, gamma2, beta2,
         trace=False):
    from concourse.bass_utils import run_bass_kernel_spmd

    N = x.shape[0]
    plan = _plan(np.asarray(edge_index), N)
    key = (N, plan["E"], plan["TOTK"], tuple(int(k) for k in plan["K"].ravel()))
    if key not in _cache:
        _cache.clear()
        _cache[key] = _compile(plan)
    nc = _cache[key]
    in_maps = _make_inputs(plan, x, W1, W2, gamma1, beta1, gamma2, beta2)
    res = run_bass_kernel_spmd(nc, in_maps, core_ids=list(range(C)), trace=trace)
    S, G = plan["S"], plan["G"]
    aT_full = np.concatenate(
        [np.asarray(res.results[c]["outT"], dtype=np.float32) for c in range(C)],
        axis=1)
    assert aT_full.shape == (128, G)
    y = np.ascontiguousarray(aT_full.T[plan["g_of_old"]], dtype=np.float32)
    return y, res


def kernel(**inputs):
    y, _ = _run(**inputs)
    return y
